# revision 1
# baseline (speedup 1.0000x reference)
"""Darknet 3x3 conv block (conv * mask + bias) on 8 TRN2 NeuronCores.

Problem: x[1,512,192,192] (*) w[512,512,3,3] stride1 pad1, then *mask + bias.

Strategy:
  - Spatial shard over H: each core computes 24 output rows x all 512 F.
  - Host packs: x zero-padded to [512,194,194], per-core slice of 26 rows,
    channel-chunked to [128, 4cc, 26, 194] bf16.  Weights transposed to
    [c_local, fm, cc, tap, f128] bf16 (tap = kh*3+kw).  Mask replicated
    across partitions as [128, 12, 384] f32.  Bias as [128, 4fm] f32.
  - Device: implicit GEMM.  Output tile [F=128, px=384] (= 2 rows x 192
    cols) accumulates 36 matmuls in PSUM (4 C-chunks x 9 taps); lhsT =
    w tile [c128, F128], rhs = shifted x window [c128, 2x192] (2D AP on
    the padded x slab - no im2col materialization).  Groups of 4 px
    tiles share one 4-bank PSUM tile -> one DVE mask-multiply, one
    ScalarE bias-add and one 768KB output DMA per group.
  - Head hiding: a few warmup matmuls on a scratch tile keep the PE busy
    (and HAM-warm) while the first input DMAs land; DMAs are split and
    ordered by first use across both HWDGE rings (x on SP, w on ACT).
  - Host unshard: concat 8 row-slabs, reshape to [1,512,192,192] f32.
"""

import sys

for _p in ("/opt/trn_rl_repo",):
    if _p not in sys.path:
        sys.path.insert(0, _p)

import numpy as np
import ml_dtypes

N_CORES = 8
C = 512
F = 512
H = 192
W = 192
HC = H // N_CORES          # output rows per core = 24
RP = HC // 2               # row-pairs per core = 12
PX = 2 * W                 # px per output tile = 384
CC = C // 128              # c chunks = 4
FM = F // 128              # f chunks = 4
TAPS = 9
NWARM = 8                  # PE warmup matmuls while first DMAs land
GRP = 4                    # px tiles per PSUM group (4 banks)

_CACHE = {}


def _build():
    import concourse.bacc as bacc
    import concourse.mybir as mybir
    from concourse.tile import TileContext

    BF = mybir.dt.bfloat16
    F32 = mybir.dt.float32

    nc = bacc.Bacc(trn_type="TRN2", num_devices=N_CORES)
    x_sh = nc.dram_tensor("x_sh", [128, CC, HC + 2, W + 2], BF, kind="ExternalInput")
    w_sh = nc.dram_tensor("w_sh", [128, FM, CC, TAPS, 128], BF, kind="ExternalInput")
    mb_sh = nc.dram_tensor("mb_sh", [128, RP * PX + FM], F32, kind="ExternalInput")
    y_sh = nc.dram_tensor("y_sh", [FM, 128, RP, PX], F32, kind="ExternalOutput")

    # x row-thirds, in first-use order: rows 0-9 cover group g=0 (+halo),
    # 10-17 cover g=1, 18-25 cover g=2.
    ROW_SPLITS = [(0, 10), (10, 16)]
    NG = RP // GRP

    with TileContext(nc) as tc:
        with (
            tc.tile_pool(name="const", bufs=1) as cpool,
            tc.tile_pool(name="psum", bufs=2, space="PSUM") as ppool,
            tc.tile_pool(name="outp", bufs=3) as opool,
        ):
            # PE warmup while the first DMAs land (HAM pre-warm + head fill)
            scratch = cpool.tile([128, PX], BF)
            nc.vector.memset(scratch[:], 0.0)
            dps = ppool.tile([128, PX], F32, name="dps", tag="ps")
            for _ in range(NWARM):
                nc.tensor.matmul(dps[:], scratch[:, :128], scratch[:],
                                 start=True, stop=True)

            xt = cpool.tile([128, CC, HC + 2, W + 2], BF)
            wt = cpool.tile([128, FM, CC, TAPS, 128], BF)
            # First-use ordered loads.  x rides the SP HWDGE ring, w/mask/b
            # ride the ACT ring, so the two streams run in parallel.
            r0, nr = ROW_SPLITS[0]
            for cc in range(CC):
                nc.scalar.dma_start(out=wt[:, 0, cc], in_=w_sh[:, 0, cc])
                nc.sync.dma_start(out=xt[:, cc, r0:r0 + nr],
                                  in_=x_sh[:, cc, r0:r0 + nr])
            for r0, nr in ROW_SPLITS[1:]:
                for cc in range(CC):
                    nc.sync.dma_start(out=xt[:, cc, r0:r0 + nr],
                                      in_=x_sh[:, cc, r0:r0 + nr])
            mbt = cpool.tile([128, RP * PX + FM], F32)
            nc.scalar.dma_start(out=mbt[:], in_=mb_sh[:])
            mt = mbt[:, :RP * PX].rearrange("p (t q) -> p t q", q=PX)
            bt = mbt[:, RP * PX:]
            for fm in range(1, FM):
                nc.scalar.dma_start(out=wt[:, fm], in_=w_sh[:, fm])

            for fm in range(FM):
                for g in range(NG):
                    last = (fm == FM - 1 and g == NG - 1)
                    if not last:
                        # one 4-bank PSUM tile holds the group's 4 output
                        # tiles.  j-outer on the first group so the first
                        # matmuls need only x rows 0-3; j-inner elsewhere
                        # (order is irrelevant at steady state).
                        pt = ppool.tile([128, GRP, 512], F32,
                                        name=f"ps_{fm}_{g}", tag="ps")
                        for j, a in ((j, a) for a in range(CC * TAPS)
                                     for j in range(GRP)):
                            cc, o = divmod(a, TAPS)
                            kh, kw = divmod(o, 3)
                            t = g * GRP + j
                            rhs = xt[:, cc, 2 * t + kh:2 * t + kh + 2, kw:kw + W]
                            nc.tensor.matmul(
                                pt[:, j, :PX], wt[:, fm, cc, o], rhs,
                                start=(a == 0), stop=(a == CC * TAPS - 1),
                            )
                        ot = opool.tile([128, GRP, PX], F32, name=f"ot_{fm}_{g}",
                                        tag="ot")
                        nc.vector.tensor_mul(ot[:], pt[:, :, :PX],
                                             mt[:, g * GRP:(g + 1) * GRP])
                        nc.scalar.activation(
                            ot[:], ot[:],
                            mybir.ActivationFunctionType.Identity,
                            bias=bt[:, fm:fm + 1],
                        )
                        nc.sync.dma_start(out=y_sh[fm, :, g * GRP:(g + 1) * GRP],
                                          in_=ot[:])
                    else:
                        # final group: merged 4-bank PSUM tile, per-tile
                        # epilogue to keep the exposed post-matmul chain short
                        pt = ppool.tile([128, GRP, 512], F32,
                                        name=f"ps_{fm}_{g}", tag="ps")
                        for j, a in ((j, a) for a in range(CC * TAPS)
                                     for j in range(GRP)):
                            cc, o = divmod(a, TAPS)
                            kh, kw = divmod(o, 3)
                            t = g * GRP + j
                            rhs = xt[:, cc, 2 * t + kh:2 * t + kh + 2, kw:kw + W]
                            nc.tensor.matmul(
                                pt[:, j, :PX], wt[:, fm, cc, o], rhs,
                                start=(a == 0), stop=(a == CC * TAPS - 1),
                            )
                        for j in range(GRP):
                            t = g * GRP + j
                            otj = opool.tile([128, PX], F32, name=f"otl_{j}",
                                             tag="otl", bufs=4)
                            nc.vector.tensor_mul(otj[:], pt[:, j, :PX], mt[:, t])
                            nc.scalar.activation(
                                otj[:], otj[:],
                                mybir.ActivationFunctionType.Identity,
                                bias=bt[:, fm:fm + 1],
                            )
                            nc.sync.dma_start(out=y_sh[fm, :, t], in_=otj[:])

    nc.compile()
    return nc


def _pack(x, w, b, mask):
    x = np.asarray(x, dtype=np.float32)
    w = np.asarray(w, dtype=np.float32)
    b = np.asarray(b, dtype=np.float32)
    mask = np.asarray(mask)

    xp = np.zeros((C, H + 2, W + 2), dtype=np.float32)
    xp[:, 1:-1, 1:-1] = x[0]
    xp = xp.astype(ml_dtypes.bfloat16)

    # [kh,kw,c,f] -> [tap, cc, c_local, fm, f128] -> [c_local, fm, cc, tap, f128]
    wt = w.transpose(2, 3, 1, 0).reshape(TAPS, CC, 128, FM, 128)
    wt = np.ascontiguousarray(wt.transpose(2, 3, 1, 0, 4)).astype(ml_dtypes.bfloat16)

    b_re = np.ascontiguousarray(b.reshape(FM, 128).T)

    mf = mask.astype(np.float32)
    in_maps = []
    for k in range(N_CORES):
        xs = xp[:, HC * k:HC * k + HC + 2, :]                 # [512, 26, 194]
        xs = np.ascontiguousarray(
            xs.reshape(CC, 128, HC + 2, W + 2).transpose(1, 0, 2, 3))
        ms = mf[HC * k:HC * k + HC].reshape(1, RP * PX)
        mb = np.concatenate(
            [np.broadcast_to(ms, (128, RP * PX)), b_re], axis=1)
        in_maps.append({"x_sh": xs, "w_sh": wt,
                        "mb_sh": np.ascontiguousarray(mb)})
    return in_maps


def _unpack(results):
    slabs = []
    for k in range(N_CORES):
        ys = results[k]["y_sh"]                               # [4, 128, 12, 384]
        slabs.append(ys.reshape(F, HC, W))
    out = np.concatenate(slabs, axis=1)                       # [512, 192, 192]
    return out[None].astype(np.float32)


def _run(inputs, **run_kwargs):
    from concourse.bass_utils import run_bass_kernel_spmd

    if "nc" not in _CACHE:
        _CACHE["nc"] = _build()
    nc = _CACHE["nc"]
    in_maps = _pack(inputs["x"], inputs["w"], inputs["b"], inputs["mask"])
    res = run_bass_kernel_spmd(nc, in_maps, core_ids=list(range(N_CORES)), **run_kwargs)
    return _unpack(res.results), res


def kernel(**inputs):
    out, _ = _run(inputs)
    return out



# revision 3
# speedup vs baseline: 1.4422x; 1.4422x over previous
"""Darknet 3x3 conv block (conv * mask + bias) on 8 TRN2 NeuronCores.

Problem: x[1,512,192,192] (*) w[512,512,3,3] stride1 pad1, then *mask + bias.

Strategy (masked gather-GEMM):
  The reference is conv(x,w)*mask + b: output pixels where mask==0 are
  exactly b, so only ~50% of output pixels need the conv.  The host
  gathers im2col columns for the mask==1 pixels only and splits them
  evenly across the 8 cores; each core runs a dense GEMM over its
  pixel list and the host scatters results back (mask==0 filled with b).
  This halves device FLOPs vs the dense conv.

  - Host packs, per core: xg [128, sum_c CC*TAPS*csz_c] bf16 - the
    gathered im2col columns, chunk-major ([chunk][cc][tap][px], chunks
    of <=512 px to match one PSUM bank).  Weights [c128, FM, CC, TAP,
    f128] bf16 (replicated).  Bias [128, FM] f32.
  - Device: per chunk, per fm: accumulate CC*TAPS=36 matmuls
    (lhsT = w tile [c128,f128], rhs = gathered xg [c128, csz]) into one
    PSUM bank, ScalarE bias-add epilogue, DMA out.  No mask multiply on
    device (all gathered pixels have mask==1).
  - DMA: x chunks split across the SP and DVE queue rings (first chunk,
    the small remainder, split 4-ways for a fast head); w + y on ACT.
  - Host unshard: scatter [512, npx] core outputs into b-filled output.
"""

import sys

for _p in ("/opt/trn_rl_repo",):
    if _p not in sys.path:
        sys.path.insert(0, _p)

import numpy as np
import ml_dtypes

N_CORES = 8
C = 512
F = 512
H = 192
W = 192
K = 3
HP = H + 2                 # padded spatial
WP = W + 2
CC = C // 128              # c chunks = 4
FM = F // 128              # f chunks = 4
TAPS = K * K
CHUNK = 512                # px per PSUM bank (2KB of f32)
NWARM = 8                  # PE warmup matmuls while first DMAs land

_CACHE = {}


def _chunks(npx):
    """Ascending chunk sizes (small first -> fast pipeline head)."""
    rem = npx % CHUNK
    out = ([rem] if rem else []) + [CHUNK] * (npx // CHUNK)
    return out


def _build(npx):
    import concourse.bacc as bacc
    import concourse.mybir as mybir
    from concourse.tile import TileContext

    BF = mybir.dt.bfloat16
    F32 = mybir.dt.float32

    chunks = _chunks(npx)
    xg_cols = CC * TAPS * npx

    nc = bacc.Bacc(trn_type="TRN2", num_devices=N_CORES)
    xg_sh = nc.dram_tensor("xg_sh", [128, xg_cols], BF, kind="ExternalInput")
    w_sh = nc.dram_tensor("w_sh", [128, FM, CC, TAPS, 128], BF, kind="ExternalInput")
    b_sh = nc.dram_tensor("b_sh", [128, FM], F32, kind="ExternalInput")
    y_sh = nc.dram_tensor("y_sh", [FM, 128, npx], F32, kind="ExternalOutput")

    with TileContext(nc) as tc:
        with (
            tc.tile_pool(name="const", bufs=1) as cpool,
            tc.tile_pool(name="pwarm", bufs=1, space="PSUM") as wpool,
            tc.tile_pool(name="psum", bufs=7, space="PSUM") as ppool,
            tc.tile_pool(name="outp", bufs=4) as opool,
            tc.tile_pool(name="xin", bufs=3) as xpool,
        ):
            # PE warmup while the first DMAs land (HAM pre-warm + head fill)
            scratch = cpool.tile([128, CHUNK], BF)
            nc.vector.memset(scratch[:], 0.0)
            dps = wpool.tile([128, CHUNK], F32, name="dps", tag="warm")
            for _ in range(NWARM):
                nc.tensor.matmul(dps[:], scratch[:, :128], scratch[:],
                                 start=True, stop=True)

            wt = cpool.tile([128, FM, CC, TAPS, 128], BF)
            bt = cpool.tile([128, FM], F32)

            # --- DMA issue, first-use order ------------------------------
            # x rides SP; w (cc-major pieces, matching the fm-interleaved
            # consumption order) + bias + y ride ACT.  First x chunk split
            # fine (cc0 by tap, then per-cc) so the PE starts early.
            xts = [xpool.tile([128, CC * TAPS * CHUNK], BF,
                              name=f"x{ci}", tag="x")
                   for ci in range(len(chunks))]
            csz0 = chunks[0]
            for t in range(TAPS):
                nc.sync.dma_start(out=xts[0][:, t * csz0:(t + 1) * csz0],
                                  in_=xg_sh[:, t * csz0:(t + 1) * csz0])
            piece = TAPS * csz0
            for cc in range(1, CC):
                nc.sync.dma_start(out=xts[0][:, cc * piece:(cc + 1) * piece],
                                  in_=xg_sh[:, cc * piece:(cc + 1) * piece])
            for cc in range(CC):
                for fm in range(FM):
                    nc.scalar.dma_start(out=wt[:, fm, cc], in_=w_sh[:, fm, cc])
                if cc == 0:
                    nc.scalar.dma_start(out=bt[:], in_=b_sh[:])
            c_off = CC * TAPS * csz0
            for ci, csz in enumerate(chunks[1:], start=1):
                n = CC * TAPS * csz
                nc.sync.dma_start(out=xts[ci][:, :n],
                                  in_=xg_sh[:, c_off:c_off + n])
                c_off += n

            # --- main loop ----------------------------------------------
            # 4 fm accumulations interleaved across 4 PSUM banks: each x
            # (cc,tap) piece is consumed by 4 matmuls as it lands, so the
            # PE tracks the DMA stream at the head instead of stalling.
            px0 = 0
            for ci, csz in enumerate(chunks):
                xv = xts[ci][:, :CC * TAPS * csz].rearrange(
                    "p (c t q) -> p c t q", c=CC, t=TAPS)
                pts = [ppool.tile([128, CHUNK], F32,
                                  name=f"ps_{ci}_{fm}", tag="ps")
                       for fm in range(FM)]
                for a in range(CC * TAPS):
                    cc, o = divmod(a, TAPS)
                    for fm in range(FM):
                        nc.tensor.matmul(
                            pts[fm][:, :csz], wt[:, fm, cc, o], xv[:, cc, o],
                            start=(a == 0), stop=(a == CC * TAPS - 1),
                        )
                for fm in range(FM):
                    ot = opool.tile([128, CHUNK], F32,
                                    name=f"ot_{ci}_{fm}", tag="ot")
                    nc.scalar.activation(
                        ot[:, :csz], pts[fm][:, :csz],
                        mybir.ActivationFunctionType.Identity,
                        bias=bt[:, fm:fm + 1],
                    )
                    nc.scalar.dma_start(out=y_sh[fm, :, px0:px0 + csz],
                                        in_=ot[:, :csz])
                px0 += csz

    nc.compile()
    return nc


def _pack(x, w, b, mask, npx, chunks):
    x = np.asarray(x, dtype=np.float32)
    w = np.asarray(w, dtype=np.float32)
    b = np.asarray(b, dtype=np.float32)
    mask = np.asarray(mask)

    xp = np.zeros((C, HP, WP), dtype=np.float32)
    xp[:, 1:-1, 1:-1] = x[0]
    xpb = xp.astype(ml_dtypes.bfloat16).reshape(CC, 128, HP * WP)

    # [kh,kw,c,f] -> [tap, cc, c_local, fm, f128] -> [c_local, fm, cc, tap, f128]
    wt = w.transpose(2, 3, 1, 0).reshape(TAPS, CC, 128, FM, 128)
    wt = np.ascontiguousarray(wt.transpose(2, 3, 1, 0, 4)).astype(ml_dtypes.bfloat16)
    b_re = np.ascontiguousarray(b.reshape(FM, 128).T)

    hs, ws = np.nonzero(mask)
    cnt = len(hs)
    total = npx * N_CORES
    # top-left of each 3x3 window in the padded image (output px (h,w)
    # reads padded rows h..h+2); pad with a repeat of the last real
    # coordinate (its duplicate output scatters the same value).
    base = hs.astype(np.int64) * WP + ws.astype(np.int64)
    if cnt == 0:
        base_pad = np.zeros(total, dtype=np.int64)
    else:
        base_pad = np.concatenate(
            [base, np.full(total - cnt, base[-1], dtype=np.int64)])
    tap_off = (np.arange(K)[:, None] * WP + np.arange(K)[None, :]).reshape(TAPS)

    bounds = []
    o = 0
    for csz in chunks:
        bounds.append((o, o + csz))
        o += csz

    in_maps = []
    for k in range(N_CORES):
        pix = base_pad[k * npx:(k + 1) * npx]
        idx = pix[None, :] + tap_off[:, None]          # [TAPS, npx]
        g = xpb[:, :, idx]                             # [CC, 128, TAPS, npx]
        g = g.transpose(1, 0, 2, 3)                    # [128, CC, TAPS, npx]
        xg = np.concatenate(
            [g[:, :, :, c0:c1].reshape(128, -1) for c0, c1 in bounds], axis=1)
        in_maps.append({"xg_sh": np.ascontiguousarray(xg), "w_sh": wt,
                        "b_sh": b_re})
    return in_maps, base_pad


def _unpack(results, b, mask, npx):
    b = np.asarray(b, dtype=np.float32)
    mask = np.asarray(mask)
    hs, ws = np.nonzero(mask)
    cnt = len(hs)

    out = np.empty((F, H * W), dtype=np.float32)
    out[:] = b[:, None]
    if cnt:
        y = np.concatenate(
            [results[k]["y_sh"].reshape(F, npx) for k in range(N_CORES)],
            axis=1)[:, :cnt]
        out[:, hs * W + ws] = y
    return out.reshape(1, F, H, W)


def _run(inputs, **run_kwargs):
    from concourse.bass_utils import run_bass_kernel_spmd

    mask = np.asarray(inputs["mask"])
    cnt = int((mask != 0).sum())
    npx = max(16, -(-cnt // N_CORES))
    npx = (npx + 15) // 16 * 16
    chunks = _chunks(npx)

    if npx not in _CACHE:
        _CACHE[npx] = _build(npx)
    nc = _CACHE[npx]
    in_maps, _ = _pack(inputs["x"], inputs["w"], inputs["b"], mask, npx, chunks)
    res = run_bass_kernel_spmd(nc, in_maps, core_ids=list(range(N_CORES)),
                               **run_kwargs)
    return _unpack(res.results, inputs["b"], mask, npx), res


def kernel(**inputs):
    out, _ = _run(inputs)
    return out


# revision 5
# speedup vs baseline: 1.7568x; 1.2182x over previous
"""Darknet 3x3 conv block (conv * mask + bias) on 8 TRN2 NeuronCores.

Problem: x[1,512,192,192] (*) w[512,512,3,3] stride1 pad1, then *mask + bias.

Strategy (masked gather-GEMM):
  The reference is conv(x,w)*mask + b: output pixels where mask==0 are
  exactly b, so only ~50% of output pixels need the conv.  The host
  gathers im2col columns for the mask==1 pixels only and splits them
  evenly across the 8 cores; each core runs a dense GEMM over its
  pixel list and the host scatters results back (mask==0 filled with b).
  This halves device FLOPs vs the dense conv.

  - Host packs, per core: xg [128, sum_c CC*TAPS*csz_c] bf16 - the
    gathered im2col columns, chunk-major ([chunk][cc][tap][px], chunks
    of <=512 px to match one PSUM bank).  Weights [c128, FM, CC, TAP,
    f128] bf16 (replicated).  Bias [128, FM] f32.
  - Device: per chunk, per fm: accumulate CC*TAPS=36 matmuls
    (lhsT = w tile [c128,f128], rhs = gathered xg [c128, csz]) into one
    PSUM bank, ScalarE bias-add epilogue, DMA out.  No mask multiply on
    device (all gathered pixels have mask==1).
  - DMA: x chunks split across the SP and DVE queue rings (first chunk,
    the small remainder, split 4-ways for a fast head); w + y on ACT.
  - Host unshard: scatter [512, npx] core outputs into b-filled output.
"""

import sys

for _p in ("/opt/trn_rl_repo",):
    if _p not in sys.path:
        sys.path.insert(0, _p)

import numpy as np
import ml_dtypes

N_CORES = 8
C = 512
F = 512
H = 192
W = 192
K = 3
HP = H + 2                 # padded spatial
WP = W + 2
CC = C // 128              # c chunks = 4
FM = F // 128              # f chunks = 4
TAPS = K * K
CHUNK = 512                # px per PSUM bank (2KB of f32)
NWARM = 8                  # PE warmup matmuls while first DMAs land

_CACHE = {}


def _chunks(npx):
    """Ascending chunk sizes (small first -> fast pipeline head)."""
    rem = npx % CHUNK
    out = ([rem] if rem else []) + [CHUNK] * (npx // CHUNK)
    return out


def _build(npx):
    import concourse.bacc as bacc
    import concourse.mybir as mybir
    from concourse.tile import TileContext

    BF = mybir.dt.bfloat16
    F32 = mybir.dt.float32

    chunks = _chunks(npx)
    xg_cols = CC * TAPS * npx

    nc = bacc.Bacc(trn_type="TRN2", num_devices=N_CORES)
    xg_sh = nc.dram_tensor("xg_sh", [128, xg_cols], BF, kind="ExternalInput")
    w_sh = nc.dram_tensor("w_sh", [128, FM, CC, TAPS, 128], BF, kind="ExternalInput")
    b_sh = nc.dram_tensor("b_sh", [128, FM], F32, kind="ExternalInput")
    y_sh = nc.dram_tensor("y_sh", [FM, 128, npx], F32, kind="ExternalOutput")

    with TileContext(nc) as tc:
        with (
            tc.tile_pool(name="const", bufs=1) as cpool,
            tc.tile_pool(name="pwarm", bufs=1, space="PSUM") as wpool,
            tc.tile_pool(name="psum", bufs=7, space="PSUM") as ppool,
            tc.tile_pool(name="outp", bufs=4) as opool,
            tc.tile_pool(name="xin", bufs=3) as xpool,
        ):
            # PE warmup while the first DMAs land (HAM pre-warm + head fill)
            scratch = cpool.tile([128, CHUNK], BF)
            nc.vector.memset(scratch[:], 0.0)
            dps = wpool.tile([128, CHUNK], F32, name="dps", tag="warm")
            for _ in range(NWARM):
                nc.tensor.matmul(dps[:], scratch[:, :128], scratch[:],
                                 start=True, stop=True)

            wt = cpool.tile([128, FM, CC, TAPS, 128], BF)
            bt = cpool.tile([128, FM], F32)

            # --- DMA issue, first-use order ------------------------------
            # x rides SP; w (cc-major pieces, matching the fm-interleaved
            # consumption order) + bias + y ride ACT.  First x chunk split
            # fine (cc0 by tap, then per-cc) so the PE starts early.
            xts = [xpool.tile([128, CC * TAPS * CHUNK], BF,
                              name=f"x{ci}", tag="x")
                   for ci in range(len(chunks))]
            csz0 = chunks[0]
            for t in range(TAPS):
                nc.sync.dma_start(out=xts[0][:, t * csz0:(t + 1) * csz0],
                                  in_=xg_sh[:, t * csz0:(t + 1) * csz0])
            piece = TAPS * csz0
            for cc in range(1, CC):
                nc.sync.dma_start(out=xts[0][:, cc * piece:(cc + 1) * piece],
                                  in_=xg_sh[:, cc * piece:(cc + 1) * piece])
            for cc in range(CC):
                nc.scalar.dma_start(out=wt[:, 0, cc], in_=w_sh[:, 0, cc])
            nc.scalar.dma_start(out=bt[:], in_=b_sh[:])
            for fm in range(1, FM):
                nc.scalar.dma_start(out=wt[:, fm], in_=w_sh[:, fm])
            c_off = CC * TAPS * csz0
            for ci, csz in enumerate(chunks[1:], start=1):
                n = CC * TAPS * csz
                nc.sync.dma_start(out=xts[ci][:, :n],
                                  in_=xg_sh[:, c_off:c_off + n])
                c_off += n

            # --- main loop ----------------------------------------------
            # Sequential 36-matmul accumulation chains (one PSUM bank per
            # (chunk, fm)): back-to-back matmuls into the same bank stream
            # at full PE rate (bank-interleaving costs ~60ns/matmul).
            px0 = 0
            for ci, csz in enumerate(chunks):
                xv = xts[ci][:, :CC * TAPS * csz].rearrange(
                    "p (c t q) -> p c t q", c=CC, t=TAPS)
                for fm in range(FM):
                    pt = ppool.tile([128, CHUNK], F32,
                                    name=f"ps_{ci}_{fm}", tag="ps")
                    for a in range(CC * TAPS):
                        cc, o = divmod(a, TAPS)
                        nc.tensor.matmul(
                            pt[:, :csz], wt[:, fm, cc, o], xv[:, cc, o],
                            start=(a == 0), stop=(a == CC * TAPS - 1),
                        )
                    ot = opool.tile([128, CHUNK], F32,
                                    name=f"ot_{ci}_{fm}", tag="ot")
                    nc.scalar.activation(
                        ot[:, :csz], pt[:, :csz],
                        mybir.ActivationFunctionType.Identity,
                        bias=bt[:, fm:fm + 1],
                    )
                    nc.scalar.dma_start(out=y_sh[fm, :, px0:px0 + csz],
                                        in_=ot[:, :csz])
                px0 += csz

    nc.compile()
    return nc


def _pack(x, w, b, mask, npx, chunks):
    x = np.asarray(x, dtype=np.float32)
    w = np.asarray(w, dtype=np.float32)
    b = np.asarray(b, dtype=np.float32)
    mask = np.asarray(mask)

    xp = np.zeros((C, HP, WP), dtype=np.float32)
    xp[:, 1:-1, 1:-1] = x[0]
    xpb = xp.astype(ml_dtypes.bfloat16).reshape(CC, 128, HP * WP)

    # [kh,kw,c,f] -> [tap, cc, c_local, fm, f128] -> [c_local, fm, cc, tap, f128]
    wt = w.transpose(2, 3, 1, 0).reshape(TAPS, CC, 128, FM, 128)
    wt = np.ascontiguousarray(wt.transpose(2, 3, 1, 0, 4)).astype(ml_dtypes.bfloat16)
    b_re = np.ascontiguousarray(b.reshape(FM, 128).T)

    hs, ws = np.nonzero(mask)
    cnt = len(hs)
    total = npx * N_CORES
    # top-left of each 3x3 window in the padded image (output px (h,w)
    # reads padded rows h..h+2); pad with a repeat of the last real
    # coordinate (its duplicate output scatters the same value).
    base = hs.astype(np.int64) * WP + ws.astype(np.int64)
    if cnt == 0:
        base_pad = np.zeros(total, dtype=np.int64)
    else:
        base_pad = np.concatenate(
            [base, np.full(total - cnt, base[-1], dtype=np.int64)])
    tap_off = (np.arange(K)[:, None] * WP + np.arange(K)[None, :]).reshape(TAPS)

    bounds = []
    o = 0
    for csz in chunks:
        bounds.append((o, o + csz))
        o += csz

    in_maps = []
    for k in range(N_CORES):
        pix = base_pad[k * npx:(k + 1) * npx]
        idx = pix[None, :] + tap_off[:, None]          # [TAPS, npx]
        g = xpb[:, :, idx]                             # [CC, 128, TAPS, npx]
        g = g.transpose(1, 0, 2, 3)                    # [128, CC, TAPS, npx]
        xg = np.concatenate(
            [g[:, :, :, c0:c1].reshape(128, -1) for c0, c1 in bounds], axis=1)
        in_maps.append({"xg_sh": np.ascontiguousarray(xg), "w_sh": wt,
                        "b_sh": b_re})
    return in_maps, base_pad


def _unpack(results, b, mask, npx):
    b = np.asarray(b, dtype=np.float32)
    mask = np.asarray(mask)
    hs, ws = np.nonzero(mask)
    cnt = len(hs)

    out = np.empty((F, H * W), dtype=np.float32)
    out[:] = b[:, None]
    if cnt:
        y = np.concatenate(
            [results[k]["y_sh"].reshape(F, npx) for k in range(N_CORES)],
            axis=1)[:, :cnt]
        out[:, hs * W + ws] = y
    return out.reshape(1, F, H, W)


def _run(inputs, **run_kwargs):
    from concourse.bass_utils import run_bass_kernel_spmd

    mask = np.asarray(inputs["mask"])
    cnt = int((mask != 0).sum())
    npx = max(16, -(-cnt // N_CORES))
    npx = (npx + 15) // 16 * 16
    chunks = _chunks(npx)

    if npx not in _CACHE:
        _CACHE[npx] = _build(npx)
    nc = _CACHE[npx]
    in_maps, _ = _pack(inputs["x"], inputs["w"], inputs["b"], mask, npx, chunks)
    res = run_bass_kernel_spmd(nc, in_maps, core_ids=list(range(N_CORES)),
                               **run_kwargs)
    return _unpack(res.results, inputs["b"], mask, npx), res


def kernel(**inputs):
    out, _ = _run(inputs)
    return out
